# revision 1
# baseline (speedup 1.0000x reference)
"""AttributeMemoryFusion kernel for 8x TRN2 NeuronCores (Bass/Tile), v6.

Per-sample attention over ragged memory + gated fusion:
    scores = mem @ h ; attn = softmax(mask(scores)) ; r = attn @ mem
    g = sigmoid(h @ Wg.T + r @ Ug.T + b) ; out = where(len>0, g*r+(1-g)*h, h)

v6 = v5 (bf16 mem/h) + length-sorted ragged packing:
  Samples are sorted by `lengths` on the host and dealt to cores so every
  core sees the same per-tile length cap profile (tile k holds samples from
  the k-th global length-octile; cap_k = that octile's max length). Only the
  first cap_k memory rows of each sample are packed and uploaded — rows
  m >= len are provably unused (masked in softmax, attn == 0 in r). With
  uniform lengths this cuts the dominant mem upload and the on-device
  HBM/dot/diag/matmul work to ~56% on top of v5's bf16 halving. The output
  permutation is undone on the host. Caps are computed from the actual
  inputs at run time (the Bass program is traced per cap profile).
"""

from contextlib import ExitStack

import numpy as np
import ml_dtypes

import concourse.bass as bass
import concourse.bacc as bacc
import concourse.mybir as mybir
import concourse.tile as tile
from concourse import masks
from concourse.bass_utils import run_bass_kernel_spmd

B, M, D = 8192, 64, 256
N_CORES = 8
BC = B // N_CORES      # samples per core
P = 128                # partitions / samples per tile
N_TILES = BC // P
BIG = 1.0e9
REPS = 1               # whole-batch repetitions (slope timing)

F32 = mybir.dt.float32
BF16 = mybir.dt.bfloat16
I32 = mybir.dt.int32
Alu = mybir.AluOpType
Act = mybir.ActivationFunctionType
AX = mybir.AxisListType


def _build_body(ctx, tc, io, caps):
    nc = tc.nc
    h_ap, mem_ap, len_ap, wg_ap, wgb_ap, ug_ap, ugb_ap, bg_ap, out_ap = io
    offs = np.concatenate([[0], np.cumsum([P * c for c in caps])])

    # ---- one-time constants ----
    const = ctx.enter_context(tc.tile_pool(name="const", bufs=1))
    ident = const.tile([P, P], F32)
    masks.make_identity(nc, ident[:])
    iota_m = const.tile([P, M], F32)
    nc.gpsimd.iota(
        iota_m[:], pattern=[[1, M]], base=0, channel_multiplier=0,
        allow_small_or_imprecise_dtypes=True,
    )
    ones_row = const.tile([1, P], BF16)
    nc.vector.memset(ones_row[:], 1.0)
    ident16 = const.tile([P, P], BF16)
    nc.vector.tensor_copy(ident16[:], ident[:])

    # ---- weights: load natural [o,i], transpose to lhsT layout [i_in, i_blk, o] ----
    wpool = ctx.enter_context(tc.tile_pool(name="weights", bufs=1))
    wg_nat = wpool.tile([P, 2, D], F32)
    ug_nat = wpool.tile([P, 2, D], F32)
    nc.sync.dma_start(wg_nat[:], wg_ap.rearrange("(a p) i -> p a i", p=P))
    nc.sync.dma_start(ug_nat[:], ug_ap.rearrange("(a p) i -> p a i", p=P))
    wgT = wpool.tile([P, 2, D], BF16)
    ugT = wpool.tile([P, 2, D], BF16)
    with tc.tile_pool(name="psw", bufs=2, space="PSUM") as psw:
        for nat, T in ((wg_nat, wgT), (ug_nat, ugT)):
            for ob in range(2):
                for ib in range(2):
                    pt = psw.tile([P, P], F32, tag="wtr")
                    nc.tensor.transpose(pt[:], nat[:, ob, ib * P:(ib + 1) * P], ident[:])
                    nc.scalar.copy(T[:, ib, ob * P:(ob + 1) * P], pt[:])

    # summed gate bias as a [1, D] bf16 row; added to the [b, o]-layout gate
    # preactivation via a rank-1 matmul (ones x bias_row)
    bt0 = wpool.tile([1, D], F32)
    bt1 = wpool.tile([1, D], F32)
    bt2 = wpool.tile([1, D], F32)
    bias_f32 = wpool.tile([1, D], F32)
    bias_row = wpool.tile([1, D], BF16)
    nc.sync.dma_start(bt0[:], wgb_ap.rearrange("(one i) -> one i", one=1))
    nc.sync.dma_start(bt1[:], ugb_ap.rearrange("(one i) -> one i", one=1))
    nc.sync.dma_start(bt2[:], bg_ap.rearrange("(one i) -> one i", one=1))
    nc.vector.tensor_add(bias_f32[:], bt0[:], bt1[:])
    nc.vector.tensor_add(bias_f32[:], bias_f32[:], bt2[:])
    nc.vector.tensor_copy(bias_row[:], bias_f32[:])
    ones_col = wpool.tile([1, P], BF16)
    nc.vector.memset(ones_col[:], 1.0)
    ones_D = wpool.tile([1, D], BF16)
    nc.vector.memset(ones_D[:], 1.0)

    # ---- pools ----
    mem_pool = ctx.enter_context(tc.tile_pool(name="mem", bufs=4))
    small = ctx.enter_context(tc.tile_pool(name="small", bufs=3))
    xstage = ctx.enter_context(tc.tile_pool(name="xstage", bufs=3))
    diag_pool = ctx.enter_context(tc.tile_pool(name="diag", bufs=16))
    out_pool = ctx.enter_context(tc.tile_pool(name="out", bufs=3))
    ps = ctx.enter_context(tc.tile_pool(name="ps", bufs=2, space="PSUM"))
    ps1 = ctx.enter_context(tc.tile_pool(name="ps1", bufs=2, space="PSUM"))

    # ---- whole-core upfront loads (tiny vs mem): h, lengths ----
    h_all = wpool.tile([P, N_TILES, D], BF16)
    nc.sync.dma_start(h_all[:], h_ap.rearrange("(t p) d -> p t d", p=P))
    lt_all = wpool.tile([P, N_TILES], I32)
    nc.sync.dma_start(lt_all[:], len_ap.rearrange("(t p) -> p t", p=P))
    lrow_all = wpool.tile([1, BC], I32)
    nc.sync.dma_start(lrow_all[:], len_ap.rearrange("(one b) -> one b", one=1))

    # prologue: per-tile +/-BIG softmax masks and empty-row gate masks
    ltf_all = wpool.tile([P, N_TILES], F32)
    nc.vector.tensor_copy(ltf_all[:], lt_all[:])
    maskbig_all = wpool.tile([P, N_TILES, M], F32)
    negrow_all = wpool.tile([1, BC], BF16)
    lrowf_all = wpool.tile([1, BC], F32)
    nc.vector.tensor_copy(lrowf_all[:], lrow_all[:])
    nc.vector.tensor_scalar(negrow_all[:], lrowf_all[:], 0.0, None, Alu.is_gt)
    nc.vector.tensor_scalar(negrow_all[:], negrow_all[:], BIG, BIG, Alu.mult, Alu.subtract)
    for t in range(N_TILES):
        mt_ = caps[t]
        nc.vector.tensor_scalar(
            maskbig_all[:, t, 0:mt_], iota_m[:, 0:mt_], ltf_all[:, t:t + 1],
            None, Alu.is_lt)
        nc.vector.tensor_scalar(
            maskbig_all[:, t, 0:mt_], maskbig_all[:, t, 0:mt_], 2.0 * BIG, BIG,
            Alu.mult, Alu.subtract)

    def scores_front(t):
        """DMA load (packed rows), scores, masked softmax, h-transpose."""
        b0 = t * P
        MT = caps[t]
        mck = mem_pool.tile([P, M, D], BF16, tag="mem")
        nc.sync.dma_start(
            mck[:, 0:MT, :],
            mem_ap[offs[t]:offs[t + 1], :].rearrange("(p m) d -> p m d", p=P),
        )

        ht = h_all[:, t, :]

        # ---- scores[b, m] = <mem[b, m, :], h[b, :]> (fused mult+accum) ----
        scratch = small.tile([P, D], BF16, tag="scratch")
        S = small.tile([P, M], F32, tag="S")
        for m in range(MT):
            nc.vector.scalar_tensor_tensor(
                out=scratch[:], in0=mck[:, m, :], scalar=1.0, in1=ht,
                op0=Alu.mult, op1=Alu.mult, accum_out=S[:, m:m + 1],
            )

        # ---- masked softmax over m: Sm = min(S, +/-BIG mask) ----
        Sm = small.tile([P, M], F32, tag="Sm")
        nc.vector.tensor_tensor(Sm[:, 0:MT], S[:, 0:MT], maskbig_all[:, t, 0:MT], Alu.min)
        negmax = small.tile([P, 1], F32, tag="negmax")
        nc.vector.tensor_reduce(negmax[:], Sm[:, 0:MT], AX.X, Alu.max, negate=True)
        E = xstage.tile([P, M], F32, tag="E")
        ssum = small.tile([P, 1], F32, tag="ssum")
        # ScalarE accumulator emits the softmax denominator with the exp
        nc.scalar.activation(E[:, 0:MT], Sm[:, 0:MT], Act.Exp, bias=negmax[:],
                             scale=1.0, accum_out=ssum[:])
        rinv = small.tile([P, 1], F32, tag="rinv")
        nc.vector.reciprocal(rinv[:], ssum[:])

        # h transpose (only needs ht)
        pt_h = ps1.tile([P, 2, P], BF16, tag="pth")
        hT = xstage.tile([P, 2, P], BF16, tag="hT")
        for k in range(2):
            nc.tensor.transpose(pt_h[:, k, :], ht[:, k * P:(k + 1) * P], ident16[:])
            nc.scalar.copy(hT[:, k, :], pt_h[:, k, :])

        return dict(ht=ht, hT=hT, negrow=negrow_all[:, b0:b0 + P],
                    attn=E, rinv=rinv, mck=mck, b0=b0, MT=MT,
                    last=(t >= N_TILES - 2))

    def r_front(st):
        """r[b, :] = sum_m attn[b, m] * mem[b, m, :], on TensorE via
        diag(attn_m) bf16 matmuls accumulated in PSUM."""
        attn, mck, MT, last = st["attn"], st["mck"], st["MT"], st["last"]
        R_ps = ps.tile([P, D], F32, tag="Rps")
        for m in range(MT):
            dg = diag_pool.tile([P, P], BF16, tag="dg")
            if last and m % 3 != 0:
                nc.vector.tensor_scalar(dg[:], ident[:], attn[:, m:m + 1], None, Alu.mult)
            else:
                nc.scalar.activation(dg[:], ident[:], Act.Copy, bias=0.0,
                                     scale=attn[:, m:m + 1])
            nc.tensor.matmul(
                R_ps[:], dg[:], mck[:, m, :],
                start=(m == 0), stop=(m == MT - 1),
            )
        st["R_ps"] = R_ps
        return st

    def backend(st):
        """Combine r, gate matmuls, sigmoid, blend, store."""
        ht, R_ps, hT, negrow, b0 = (
            st["ht"], st["R_ps"], st["hT"], st["negrow"], st["b0"]
        )
        R = small.tile([P, D], F32, tag="R")
        nc.scalar.activation(R[:], R_ps[:], Act.Copy, bias=0.0, scale=st["rinv"][:])
        Rb = small.tile([P, D], BF16, tag="Rb")
        nc.vector.tensor_copy(Rb[:], R[:])

        pt_r = ps1.tile([P, 2, P], BF16, tag="ptr")
        rT = small.tile([P, 2, P], BF16, tag="rT")
        for k in range(2):
            nc.tensor.transpose(pt_r[:, k, :], Rb[:, k * P:(k + 1) * P], ident16[:])
            nc.scalar.copy(rT[:, k, :], pt_r[:, k, :])

        # ---- gate preactivation directly in [b, o] layout ----
        # G[b, o] = sum_d hT[d, b] Wg^T[d, o] + sum_d rT[d, b] Ug^T[d, o]
        #           + bias[o] + (-BIG if len_b == 0)
        # (contraction over d: lhsT = hT/rT blocks, rhs = wgT/ugT blocks;
        #  bias and empty-row mask enter as rank-1 matmuls)
        G = ps.tile([P, D], F32, tag="G")
        for ib in range(2):
            nc.tensor.matmul(G[:], hT[:, ib, :], wgT[:, ib, :],
                             start=(ib == 0), stop=False)
        for ib in range(2):
            nc.tensor.matmul(G[:], rT[:, ib, :], ugT[:, ib, :],
                             start=False, stop=False)
        nc.tensor.matmul(G[:], ones_col[:], bias_row[:], start=False, stop=False)
        nc.tensor.matmul(G[:], negrow[:], ones_D[:], start=False, stop=True)

        # y = tanh((pre + bias)/2); g = 0.5*(1+y) folded into the blend.
        g_sb = small.tile([P, D], F32, tag="gT")
        nc.scalar.activation(g_sb[:], G[:], Act.Tanh, bias=0.0, scale=0.5)

        # ---- out = h + 0.5*(1+y)*(r-h) ----
        T1 = small.tile([P, D], F32, tag="T1")
        nc.vector.tensor_tensor(T1[:], R[:], ht, Alu.subtract)
        T2 = small.tile([P, D], F32, tag="T2")
        nc.vector.scalar_tensor_tensor(
            out=T2[:], in0=g_sb[:], scalar=1.0,
            in1=T1[:], op0=Alu.add, op1=Alu.mult,
        )
        ot = out_pool.tile([P, D], F32, tag="ot")
        nc.vector.scalar_tensor_tensor(
            out=ot[:], in0=T2[:], scalar=0.5, in1=ht, op0=Alu.mult, op1=Alu.add,
        )
        nc.sync.dma_start(out_ap[b0:b0 + P, :], ot[:])

    # 3-stage software pipeline.
    for _rep in range(REPS):
        stages = []
        for t in range(N_TILES):
            stages.append(scores_front(t))
            if t >= 1:
                r_front(stages[t - 1])
            if t >= 2:
                backend(stages[t - 2])
        r_front(stages[N_TILES - 1])
        backend(stages[N_TILES - 2])
        backend(stages[N_TILES - 1])


_CACHE = {}


def _get_nc(caps):
    key = ("nc", REPS, caps)
    if key in _CACHE:
        return _CACHE[key]
    total_rows = int(P * sum(caps))
    nc = bacc.Bacc("TRN2", target_bir_lowering=False, debug=False, num_devices=N_CORES)
    h_ap = nc.dram_tensor("h_tilde", [BC, D], BF16, kind="ExternalInput").ap()
    mem_ap = nc.dram_tensor("mem", [total_rows, D], BF16, kind="ExternalInput").ap()
    len_ap = nc.dram_tensor("lengths", [BC], I32, kind="ExternalInput").ap()
    wg_ap = nc.dram_tensor("Wg_w", [D, D], F32, kind="ExternalInput").ap()
    wgb_ap = nc.dram_tensor("Wg_b", [D], F32, kind="ExternalInput").ap()
    ug_ap = nc.dram_tensor("Ug_w", [D, D], F32, kind="ExternalInput").ap()
    ugb_ap = nc.dram_tensor("Ug_b", [D], F32, kind="ExternalInput").ap()
    bg_ap = nc.dram_tensor("b_g", [D], F32, kind="ExternalInput").ap()
    out_ap = nc.dram_tensor("out", [BC, D], F32, kind="ExternalOutput").ap()
    io = (h_ap, mem_ap, len_ap, wg_ap, wgb_ap, ug_ap, ugb_ap, bg_ap, out_ap)
    with tile.TileContext(nc) as tc:
        with ExitStack() as ctx:
            _build_body(ctx, tc, io, caps)
    nc.finalize()
    _CACHE[key] = nc
    return nc


def _plan(lengths):
    """Sort samples by length; deal global octile blocks across cores so
    every core has the same per-tile cap profile. Returns (perm[B] of
    sample ids in device order core-major, caps[N_TILES])."""
    order = np.argsort(lengths, kind="stable")
    caps = []
    perm = np.empty(B, dtype=np.int64)
    # Tile scheduling order over the ascending length octiles: small tiles
    # at both ends so neither the pipeline fill (tile 0) nor the drain
    # (last tile) exposes a long serial chain.
    import os
    _ord = os.environ.get("K_TILE_ORDER")
    if _ord:
        tile_order = [int(x) for x in _ord.split(",")]
    else:
        tile_order = list(range(N_TILES))
    for k in range(N_TILES):
        kk = tile_order[k]
        blk = order[kk * (P * N_CORES):(kk + 1) * (P * N_CORES)]
        caps.append(int(max(1, lengths[blk].max())))
        # core c, tile k, partition p <- blk[p * N_CORES + c]
        for c in range(N_CORES):
            perm[c * BC + k * P: c * BC + (k + 1) * P] = blk[c::N_CORES]
    return perm, tuple(caps)


def _make_in_maps(inputs):
    lengths_full = np.asarray(inputs["lengths"], dtype=np.int32)
    perm, caps = _plan(lengths_full)
    h = np.asarray(inputs["h_tilde"], dtype=np.float32).astype(ml_dtypes.bfloat16)
    mem = np.asarray(inputs["mem"])
    shared = {
        "Wg_w": np.ascontiguousarray(np.asarray(inputs["Wg_w"], dtype=np.float32)),
        "Wg_b": np.ascontiguousarray(np.asarray(inputs["Wg_b"], dtype=np.float32)),
        "Ug_w": np.ascontiguousarray(np.asarray(inputs["Ug_w"], dtype=np.float32)),
        "Ug_b": np.ascontiguousarray(np.asarray(inputs["Ug_b"], dtype=np.float32)),
        "b_g": np.ascontiguousarray(np.asarray(inputs["b_g"], dtype=np.float32)),
    }
    in_maps = []
    for c in range(N_CORES):
        ids = perm[c * BC:(c + 1) * BC]
        # slice first, cast second: only the ~56% packed rows get converted
        packed = np.concatenate([
            np.ascontiguousarray(mem[ids[k * P:(k + 1) * P], :caps[k], :])
            .astype(ml_dtypes.bfloat16).reshape(P * caps[k], D)
            for k in range(N_TILES)
        ], axis=0)
        in_maps.append({
            "h_tilde": np.ascontiguousarray(h[ids]),
            "mem": np.ascontiguousarray(packed),
            "lengths": np.ascontiguousarray(lengths_full[ids]),
            **shared,
        })
    return in_maps, perm, caps


def run(inputs, **kwargs):
    in_maps, perm, caps = _make_in_maps(inputs)
    nc = _get_nc(caps)
    res = run_bass_kernel_spmd(nc, in_maps, list(range(N_CORES)), **kwargs)
    return res, perm


def kernel(**inputs) -> np.ndarray:
    res, perm = run(inputs)
    permuted = np.concatenate(
        [res.results[c]["out"] for c in range(N_CORES)], axis=0)
    out = np.empty_like(permuted)
    out[perm] = permuted
    return out



# revision 3
# speedup vs baseline: 1.8095x; 1.8095x over previous
"""AttributeMemoryFusion kernel for 8x TRN2 NeuronCores (Bass/Tile), v7.

Per-sample attention over ragged memory + gated fusion:
    scores = mem @ h ; attn = softmax(mask(scores)) ; r = attn @ mem
    g = sigmoid(h @ Wg.T + r @ Ug.T + b) ; out = where(len>0, g*r+(1-g)*h, h)

v7 = v6 (length-sorted ragged packing) + int8 row-quantized mem transport.
  The wall-clock of a kernel() call here is dominated by the ~80 MB/s axon
  host->device tunnel, so mem rows are shipped as int8 `q` with a per-row
  f32 `scale` (q = round(mem_row / scale), scale = absmax/127) instead of
  bf16 — halving the dominant payload. On device q is cast once per tile to
  bf16 (integers <= 127 are exact in bf16); `scale` is folded into the
  scores before the softmax and into the exp weights before the attn @ mem
  matmul, so no per-row dequantization pass is needed and the compute
  pipeline is unchanged from v6. The output returns as bf16 to halve the
  device->host payload too.
"""

from contextlib import ExitStack

import numpy as np
import ml_dtypes

import concourse.bass as bass
import concourse.bacc as bacc
import concourse.mybir as mybir
import concourse.tile as tile
from concourse import masks
from concourse.bass_utils import run_bass_kernel_spmd

B, M, D = 8192, 64, 256
N_CORES = 8
BC = B // N_CORES      # samples per core
P = 128                # partitions / samples per tile
N_TILES = BC // P
BIG = 1.0e9

F32 = mybir.dt.float32
BF16 = mybir.dt.bfloat16
I32 = mybir.dt.int32
I8 = mybir.dt.int8
Alu = mybir.AluOpType
Act = mybir.ActivationFunctionType
AX = mybir.AxisListType


def _build_body(ctx, tc, io, caps):
    nc = tc.nc
    (h_ap, mem_ap, sc_ap, len_ap, wg_ap, wgb_ap, ug_ap, ugb_ap, bg_ap,
     out_ap) = io
    offs = np.concatenate([[0], np.cumsum([P * c for c in caps])])

    # ---- one-time constants ----
    const = ctx.enter_context(tc.tile_pool(name="const", bufs=1))
    ident = const.tile([P, P], F32)
    masks.make_identity(nc, ident[:])
    iota_m = const.tile([P, M], F32)
    nc.gpsimd.iota(
        iota_m[:], pattern=[[1, M]], base=0, channel_multiplier=0,
        allow_small_or_imprecise_dtypes=True,
    )
    ident16 = const.tile([P, P], BF16)
    nc.vector.tensor_copy(ident16[:], ident[:])

    # ---- weights: load natural [o,i], transpose to lhsT layout [i_in, i_blk, o] ----
    wpool = ctx.enter_context(tc.tile_pool(name="weights", bufs=1))
    wg_nat = wpool.tile([P, 2, D], F32)
    ug_nat = wpool.tile([P, 2, D], F32)
    nc.sync.dma_start(wg_nat[:], wg_ap.rearrange("(a p) i -> p a i", p=P))
    nc.sync.dma_start(ug_nat[:], ug_ap.rearrange("(a p) i -> p a i", p=P))
    wgT = wpool.tile([P, 2, D], BF16)
    ugT = wpool.tile([P, 2, D], BF16)
    with tc.tile_pool(name="psw", bufs=2, space="PSUM") as psw:
        for nat, T in ((wg_nat, wgT), (ug_nat, ugT)):
            for ob in range(2):
                for ib in range(2):
                    pt = psw.tile([P, P], F32, tag="wtr")
                    nc.tensor.transpose(pt[:], nat[:, ob, ib * P:(ib + 1) * P], ident[:])
                    nc.scalar.copy(T[:, ib, ob * P:(ob + 1) * P], pt[:])

    # summed gate bias as a [1, D] bf16 row; added to the [b, o]-layout gate
    # preactivation via a rank-1 matmul (ones x bias_row)
    bt0 = wpool.tile([1, D], F32)
    bt1 = wpool.tile([1, D], F32)
    bt2 = wpool.tile([1, D], F32)
    bias_f32 = wpool.tile([1, D], F32)
    bias_row = wpool.tile([1, D], BF16)
    nc.sync.dma_start(bt0[:], wgb_ap.rearrange("(one i) -> one i", one=1))
    nc.sync.dma_start(bt1[:], ugb_ap.rearrange("(one i) -> one i", one=1))
    nc.sync.dma_start(bt2[:], bg_ap.rearrange("(one i) -> one i", one=1))
    nc.vector.tensor_add(bias_f32[:], bt0[:], bt1[:])
    nc.vector.tensor_add(bias_f32[:], bias_f32[:], bt2[:])
    nc.vector.tensor_copy(bias_row[:], bias_f32[:])
    ones_col = wpool.tile([1, P], BF16)
    nc.vector.memset(ones_col[:], 1.0)
    ones_D = wpool.tile([1, D], BF16)
    nc.vector.memset(ones_D[:], 1.0)

    # ---- pools ----
    memq_pool = ctx.enter_context(tc.tile_pool(name="memq", bufs=3))
    mem_pool = ctx.enter_context(tc.tile_pool(name="mem", bufs=3))
    small = ctx.enter_context(tc.tile_pool(name="small", bufs=3))
    xstage = ctx.enter_context(tc.tile_pool(name="xstage", bufs=3))
    diag_pool = ctx.enter_context(tc.tile_pool(name="diag", bufs=16))
    out_pool = ctx.enter_context(tc.tile_pool(name="out", bufs=3))
    ps = ctx.enter_context(tc.tile_pool(name="ps", bufs=2, space="PSUM"))
    ps1 = ctx.enter_context(tc.tile_pool(name="ps1", bufs=2, space="PSUM"))

    # ---- whole-core upfront loads (tiny vs mem): h, lengths ----
    h_all = wpool.tile([P, N_TILES, D], BF16)
    nc.sync.dma_start(h_all[:], h_ap.rearrange("(t p) d -> p t d", p=P))
    lt_all = wpool.tile([P, N_TILES], I32)
    nc.sync.dma_start(lt_all[:], len_ap.rearrange("(t p) -> p t", p=P))
    lrow_all = wpool.tile([1, BC], I32)
    nc.sync.dma_start(lrow_all[:], len_ap.rearrange("(one b) -> one b", one=1))

    # prologue: per-tile +/-BIG softmax masks and empty-row gate masks
    ltf_all = wpool.tile([P, N_TILES], F32)
    nc.vector.tensor_copy(ltf_all[:], lt_all[:])
    maskbig_all = wpool.tile([P, N_TILES, M], F32)
    negrow_all = wpool.tile([1, BC], BF16)
    lrowf_all = wpool.tile([1, BC], F32)
    nc.vector.tensor_copy(lrowf_all[:], lrow_all[:])
    nc.vector.tensor_scalar(negrow_all[:], lrowf_all[:], 0.0, None, Alu.is_gt)
    nc.vector.tensor_scalar(negrow_all[:], negrow_all[:], BIG, BIG, Alu.mult, Alu.subtract)
    for t in range(N_TILES):
        mt_ = caps[t]
        nc.vector.tensor_scalar(
            maskbig_all[:, t, 0:mt_], iota_m[:, 0:mt_], ltf_all[:, t:t + 1],
            None, Alu.is_lt)
        nc.vector.tensor_scalar(
            maskbig_all[:, t, 0:mt_], maskbig_all[:, t, 0:mt_], 2.0 * BIG, BIG,
            Alu.mult, Alu.subtract)

    def scores_front(t):
        """DMA load (packed int8 rows + scales), cast, scores, masked
        softmax, h-transpose."""
        b0 = t * P
        MT = caps[t]
        mq = memq_pool.tile([P, M, D], I8, tag="memq")
        nc.sync.dma_start(
            mq[:, 0:MT, :],
            mem_ap[offs[t]:offs[t + 1], :].rearrange("(p m) d -> p m d", p=P),
        )
        sc = small.tile([P, M], F32, tag="sc")
        nc.sync.dma_start(
            sc[:, 0:MT],
            sc_ap[offs[t]:offs[t + 1]].rearrange("(p m) -> p m", p=P),
        )
        # cast q -> bf16 (integers <= 127 are exact in bf16)
        mck = mem_pool.tile([P, M, D], BF16, tag="mem")
        nc.vector.tensor_copy(mck[:, 0:MT, :], mq[:, 0:MT, :])

        ht = h_all[:, t, :]

        # ---- scores[b, m] = <q[b, m, :], h[b, :]> (fused mult+accum) ----
        scratch = small.tile([P, D], BF16, tag="scratch")
        S = small.tile([P, M], F32, tag="S")
        for m in range(MT):
            nc.vector.scalar_tensor_tensor(
                out=scratch[:], in0=mck[:, m, :], scalar=1.0, in1=ht,
                op0=Alu.mult, op1=Alu.mult, accum_out=S[:, m:m + 1],
            )
        # fold the per-row dequant scale into the scores (pre-softmax)
        nc.vector.tensor_tensor(S[:, 0:MT], S[:, 0:MT], sc[:, 0:MT], Alu.mult)

        # ---- masked softmax over m: Sm = min(S, +/-BIG mask) ----
        Sm = small.tile([P, M], F32, tag="Sm")
        nc.vector.tensor_tensor(Sm[:, 0:MT], S[:, 0:MT], maskbig_all[:, t, 0:MT], Alu.min)
        negmax = small.tile([P, 1], F32, tag="negmax")
        nc.vector.tensor_reduce(negmax[:], Sm[:, 0:MT], AX.X, Alu.max, negate=True)
        E = xstage.tile([P, M], F32, tag="E")
        ssum = small.tile([P, 1], F32, tag="ssum")
        # ScalarE accumulator emits the softmax denominator with the exp
        nc.scalar.activation(E[:, 0:MT], Sm[:, 0:MT], Act.Exp, bias=negmax[:],
                             scale=1.0, accum_out=ssum[:])
        rinv = small.tile([P, 1], F32, tag="rinv")
        nc.vector.reciprocal(rinv[:], ssum[:])
        # fold the dequant scale into the attention weights for r = attn @ mem
        nc.vector.tensor_tensor(E[:, 0:MT], E[:, 0:MT], sc[:, 0:MT], Alu.mult)

        # h transpose (only needs ht)
        pt_h = ps1.tile([P, 2, P], BF16, tag="pth")
        hT = xstage.tile([P, 2, P], BF16, tag="hT")
        for k in range(2):
            nc.tensor.transpose(pt_h[:, k, :], ht[:, k * P:(k + 1) * P], ident16[:])
            nc.scalar.copy(hT[:, k, :], pt_h[:, k, :])

        return dict(ht=ht, hT=hT, negrow=negrow_all[:, b0:b0 + P],
                    attn=E, rinv=rinv, mck=mck, b0=b0, MT=MT,
                    last=(t >= N_TILES - 2))

    def r_front(st):
        """r[b, :] = sum_m attn'[b, m] * q[b, m, :], on TensorE via
        diag(attn'_m) bf16 matmuls accumulated in PSUM."""
        attn, mck, MT, last = st["attn"], st["mck"], st["MT"], st["last"]
        R_ps = ps.tile([P, D], F32, tag="Rps")
        for m in range(MT):
            dg = diag_pool.tile([P, P], BF16, tag="dg")
            if last and m % 3 != 0:
                nc.vector.tensor_scalar(dg[:], ident[:], attn[:, m:m + 1], None, Alu.mult)
            else:
                nc.scalar.activation(dg[:], ident[:], Act.Copy, bias=0.0,
                                     scale=attn[:, m:m + 1])
            nc.tensor.matmul(
                R_ps[:], dg[:], mck[:, m, :],
                start=(m == 0), stop=(m == MT - 1),
            )
        st["R_ps"] = R_ps
        return st

    def backend(st):
        """Combine r, gate matmuls, sigmoid, blend, store."""
        ht, R_ps, hT, negrow, b0 = (
            st["ht"], st["R_ps"], st["hT"], st["negrow"], st["b0"]
        )
        R = small.tile([P, D], F32, tag="R")
        nc.scalar.activation(R[:], R_ps[:], Act.Copy, bias=0.0, scale=st["rinv"][:])
        Rb = small.tile([P, D], BF16, tag="Rb")
        nc.vector.tensor_copy(Rb[:], R[:])

        pt_r = ps1.tile([P, 2, P], BF16, tag="ptr")
        rT = small.tile([P, 2, P], BF16, tag="rT")
        for k in range(2):
            nc.tensor.transpose(pt_r[:, k, :], Rb[:, k * P:(k + 1) * P], ident16[:])
            nc.scalar.copy(rT[:, k, :], pt_r[:, k, :])

        # ---- gate preactivation directly in [b, o] layout ----
        # G[b, o] = sum_d hT[d, b] Wg^T[d, o] + sum_d rT[d, b] Ug^T[d, o]
        #           + bias[o] + (-BIG if len_b == 0)
        # (contraction over d: lhsT = hT/rT blocks, rhs = wgT/ugT blocks;
        #  bias and empty-row mask enter as rank-1 matmuls)
        G = ps.tile([P, D], F32, tag="G")
        for ib in range(2):
            nc.tensor.matmul(G[:], hT[:, ib, :], wgT[:, ib, :],
                             start=(ib == 0), stop=False)
        for ib in range(2):
            nc.tensor.matmul(G[:], rT[:, ib, :], ugT[:, ib, :],
                             start=False, stop=False)
        nc.tensor.matmul(G[:], ones_col[:], bias_row[:], start=False, stop=False)
        nc.tensor.matmul(G[:], negrow[:], ones_D[:], start=False, stop=True)

        # y = tanh((pre + bias)/2); g = 0.5*(1+y) folded into the blend.
        g_sb = small.tile([P, D], F32, tag="gT")
        nc.scalar.activation(g_sb[:], G[:], Act.Tanh, bias=0.0, scale=0.5)

        # ---- out = h + 0.5*(1+y)*(r-h) ----
        T1 = small.tile([P, D], F32, tag="T1")
        nc.vector.tensor_tensor(T1[:], R[:], ht, Alu.subtract)
        T2 = small.tile([P, D], F32, tag="T2")
        nc.vector.scalar_tensor_tensor(
            out=T2[:], in0=g_sb[:], scalar=1.0,
            in1=T1[:], op0=Alu.add, op1=Alu.mult,
        )
        ot = out_pool.tile([P, D], BF16, tag="ot")
        nc.vector.scalar_tensor_tensor(
            out=ot[:], in0=T2[:], scalar=0.5, in1=ht, op0=Alu.mult, op1=Alu.add,
        )
        nc.sync.dma_start(out_ap[b0:b0 + P, :], ot[:])

    # 3-stage software pipeline.
    stages = []
    for t in range(N_TILES):
        stages.append(scores_front(t))
        if t >= 1:
            r_front(stages[t - 1])
        if t >= 2:
            backend(stages[t - 2])
    r_front(stages[N_TILES - 1])
    backend(stages[N_TILES - 2])
    backend(stages[N_TILES - 1])


_CACHE = {}


def _get_nc(caps):
    key = ("nc", caps)
    if key in _CACHE:
        return _CACHE[key]
    total_rows = int(P * sum(caps))
    nc = bacc.Bacc("TRN2", target_bir_lowering=False, debug=False, num_devices=N_CORES)
    h_ap = nc.dram_tensor("h_tilde", [BC, D], BF16, kind="ExternalInput").ap()
    mem_ap = nc.dram_tensor("mem", [total_rows, D], I8, kind="ExternalInput").ap()
    sc_ap = nc.dram_tensor("scales", [total_rows], F32, kind="ExternalInput").ap()
    len_ap = nc.dram_tensor("lengths", [BC], I32, kind="ExternalInput").ap()
    wg_ap = nc.dram_tensor("Wg_w", [D, D], F32, kind="ExternalInput").ap()
    wgb_ap = nc.dram_tensor("Wg_b", [D], F32, kind="ExternalInput").ap()
    ug_ap = nc.dram_tensor("Ug_w", [D, D], F32, kind="ExternalInput").ap()
    ugb_ap = nc.dram_tensor("Ug_b", [D], F32, kind="ExternalInput").ap()
    bg_ap = nc.dram_tensor("b_g", [D], F32, kind="ExternalInput").ap()
    out_ap = nc.dram_tensor("out", [BC, D], BF16, kind="ExternalOutput").ap()
    io = (h_ap, mem_ap, sc_ap, len_ap, wg_ap, wgb_ap, ug_ap, ugb_ap, bg_ap,
          out_ap)
    with tile.TileContext(nc) as tc:
        with ExitStack() as ctx:
            _build_body(ctx, tc, io, caps)
    nc.finalize()
    _CACHE[key] = nc
    return nc


def _plan(lengths):
    """Sort samples by length; deal global octile blocks across cores so
    every core has the same per-tile cap profile. Returns (perm[B] of
    sample ids in device order core-major, caps[N_TILES])."""
    order = np.argsort(lengths, kind="stable")
    caps = []
    perm = np.empty(B, dtype=np.int64)
    for k in range(N_TILES):
        blk = order[k * (P * N_CORES):(k + 1) * (P * N_CORES)]
        caps.append(int(max(1, lengths[blk].max())))
        # core c, tile k, partition p <- blk[p * N_CORES + c]
        for c in range(N_CORES):
            perm[c * BC + k * P: c * BC + (k + 1) * P] = blk[c::N_CORES]
    return perm, tuple(caps)


def _make_in_maps(inputs):
    lengths_full = np.asarray(inputs["lengths"], dtype=np.int32)
    perm, caps = _plan(lengths_full)
    h = np.asarray(inputs["h_tilde"], dtype=np.float32).astype(ml_dtypes.bfloat16)
    mem = np.asarray(inputs["mem"])
    shared = {
        "Wg_w": np.ascontiguousarray(np.asarray(inputs["Wg_w"], dtype=np.float32)),
        "Wg_b": np.ascontiguousarray(np.asarray(inputs["Wg_b"], dtype=np.float32)),
        "Ug_w": np.ascontiguousarray(np.asarray(inputs["Ug_w"], dtype=np.float32)),
        "Ug_b": np.ascontiguousarray(np.asarray(inputs["Ug_b"], dtype=np.float32)),
        "b_g": np.ascontiguousarray(np.asarray(inputs["b_g"], dtype=np.float32)),
    }
    total_rows = int(P * sum(caps))

    def pack_core(c):
        ids = perm[c * BC:(c + 1) * BC]
        q_parts = np.empty((total_rows, D), dtype=np.int8)
        s_parts = np.empty((total_rows,), dtype=np.float32)
        off = 0
        for k in range(N_TILES):
            blk = np.ascontiguousarray(mem[ids[k * P:(k + 1) * P], :caps[k], :],
                                       dtype=np.float32)
            rows = blk.reshape(P * caps[k], D)
            amax = np.abs(rows).max(axis=1)
            np.maximum(amax, 1e-30, out=amax)
            scale = amax * (1.0 / 127.0)
            q = np.rint(rows * (1.0 / scale)[:, None])
            q_parts[off:off + rows.shape[0]] = q
            s_parts[off:off + rows.shape[0]] = scale
            off += rows.shape[0]
        return {
            "h_tilde": np.ascontiguousarray(h[ids]),
            "mem": q_parts,
            "scales": s_parts,
            "lengths": np.ascontiguousarray(lengths_full[ids]),
            **shared,
        }

    from concurrent.futures import ThreadPoolExecutor
    with ThreadPoolExecutor(max_workers=N_CORES) as ex:
        in_maps = list(ex.map(pack_core, range(N_CORES)))
    return in_maps, perm, caps


def run(inputs, **kwargs):
    in_maps, perm, caps = _make_in_maps(inputs)
    nc = _get_nc(caps)
    res = run_bass_kernel_spmd(nc, in_maps, list(range(N_CORES)), **kwargs)
    return res, perm


def kernel(**inputs) -> np.ndarray:
    res, perm = run(inputs)
    permuted = np.concatenate(
        [res.results[c]["out"] for c in range(N_CORES)], axis=0)
    out = np.empty((B, D), dtype=np.float32)
    out[perm] = permuted
    return out


# revision 8
# speedup vs baseline: 2.3162x; 1.2801x over previous
"""AttributeMemoryFusion kernel for 8x TRN2 NeuronCores (Bass/Tile), v7.

Per-sample attention over ragged memory + gated fusion:
    scores = mem @ h ; attn = softmax(mask(scores)) ; r = attn @ mem
    g = sigmoid(h @ Wg.T + r @ Ug.T + b) ; out = where(len>0, g*r+(1-g)*h, h)

v7 = v6 (length-sorted ragged packing) + int8 row-quantized mem transport.
  The wall-clock of a kernel() call here is dominated by the ~80 MB/s axon
  host->device tunnel, so mem rows are shipped as int8 `q` with a per-row
  f32 `scale` (q = round(mem_row / scale), scale = absmax/127) instead of
  bf16 — halving the dominant payload. On device q is cast once per tile to
  bf16 (integers <= 127 are exact in bf16); `scale` is folded into the
  scores before the softmax and into the exp weights before the attn @ mem
  matmul, so no per-row dequantization pass is needed and the compute
  pipeline is unchanged from v6. The output returns as bf16 to halve the
  device->host payload too.
"""

from contextlib import ExitStack

import numpy as np
import ml_dtypes

import concourse.bass as bass
import concourse.bacc as bacc
import concourse.mybir as mybir
import concourse.tile as tile
from concourse import masks
from concourse.bass_utils import run_bass_kernel_spmd

B, M, D = 8192, 64, 256
N_CORES = 8
BC = B // N_CORES      # samples per core
P = 128                # partitions / samples per tile
N_TILES = BC // P
BIG = 1.0e9

F32 = mybir.dt.float32
F16 = mybir.dt.float16
BF16 = mybir.dt.bfloat16
I32 = mybir.dt.int32
I8 = mybir.dt.int8
Alu = mybir.AluOpType
Act = mybir.ActivationFunctionType
AX = mybir.AxisListType


def _build_body(ctx, tc, io, caps):
    nc = tc.nc
    (h_ap, mem_ap, sc_ap, len_ap, wg_ap, wgb_ap, ug_ap, ugb_ap, bg_ap,
     out_ap) = io
    offs = np.concatenate([[0], np.cumsum([P * c for c in caps])])

    # ---- one-time constants ----
    const = ctx.enter_context(tc.tile_pool(name="const", bufs=1))
    ident = const.tile([P, P], F32)
    masks.make_identity(nc, ident[:])
    iota_m = const.tile([P, M], F32)
    nc.gpsimd.iota(
        iota_m[:], pattern=[[1, M]], base=0, channel_multiplier=0,
        allow_small_or_imprecise_dtypes=True,
    )
    ident16 = const.tile([P, P], BF16)
    nc.vector.tensor_copy(ident16[:], ident[:])

    # ---- weights (shipped bf16): load natural [o,i], transpose to lhsT
    # layout [i_in, i_blk, o] ----
    wpool = ctx.enter_context(tc.tile_pool(name="weights", bufs=1))
    wg_nat = wpool.tile([P, 2, D], BF16)
    ug_nat = wpool.tile([P, 2, D], BF16)
    nc.sync.dma_start(wg_nat[:], wg_ap.rearrange("(a p) i -> p a i", p=P))
    nc.sync.dma_start(ug_nat[:], ug_ap.rearrange("(a p) i -> p a i", p=P))
    wgT = wpool.tile([P, 2, D], BF16)
    ugT = wpool.tile([P, 2, D], BF16)
    with tc.tile_pool(name="psw", bufs=2, space="PSUM") as psw:
        for nat, T in ((wg_nat, wgT), (ug_nat, ugT)):
            for ob in range(2):
                for ib in range(2):
                    pt = psw.tile([P, P], BF16, tag="wtr")
                    nc.tensor.transpose(pt[:], nat[:, ob, ib * P:(ib + 1) * P], ident16[:])
                    nc.scalar.copy(T[:, ib, ob * P:(ob + 1) * P], pt[:])

    # summed gate bias as a [1, D] bf16 row; added to the [b, o]-layout gate
    # preactivation via a rank-1 matmul (ones x bias_row)
    bt0 = wpool.tile([1, D], F32)
    bt1 = wpool.tile([1, D], F32)
    bt2 = wpool.tile([1, D], F32)
    bias_f32 = wpool.tile([1, D], F32)
    bias_row = wpool.tile([1, D], BF16)
    nc.sync.dma_start(bt0[:], wgb_ap.rearrange("(one i) -> one i", one=1))
    nc.sync.dma_start(bt1[:], ugb_ap.rearrange("(one i) -> one i", one=1))
    nc.sync.dma_start(bt2[:], bg_ap.rearrange("(one i) -> one i", one=1))
    nc.vector.tensor_add(bias_f32[:], bt0[:], bt1[:])
    nc.vector.tensor_add(bias_f32[:], bias_f32[:], bt2[:])
    nc.vector.tensor_copy(bias_row[:], bias_f32[:])
    ones_col = wpool.tile([1, P], BF16)
    nc.vector.memset(ones_col[:], 1.0)
    ones_D = wpool.tile([1, D], BF16)
    nc.vector.memset(ones_D[:], 1.0)

    # ---- pools ----
    memq_pool = ctx.enter_context(tc.tile_pool(name="memq", bufs=3))
    mem_pool = ctx.enter_context(tc.tile_pool(name="mem", bufs=3))
    small = ctx.enter_context(tc.tile_pool(name="small", bufs=3))
    xstage = ctx.enter_context(tc.tile_pool(name="xstage", bufs=3))
    diag_pool = ctx.enter_context(tc.tile_pool(name="diag", bufs=16))
    out_pool = ctx.enter_context(tc.tile_pool(name="out", bufs=3))
    ps = ctx.enter_context(tc.tile_pool(name="ps", bufs=2, space="PSUM"))
    ps1 = ctx.enter_context(tc.tile_pool(name="ps1", bufs=2, space="PSUM"))

    # ---- whole-core upfront loads (tiny vs mem): h, lengths ----
    h_all = wpool.tile([P, N_TILES, D], BF16)
    nc.sync.dma_start(h_all[:], h_ap.rearrange("(t p) d -> p t d", p=P))
    lt_all = wpool.tile([P, N_TILES], I32)
    nc.sync.dma_start(lt_all[:], len_ap.rearrange("(t p) -> p t", p=P))
    lrow_all = wpool.tile([1, BC], I32)
    nc.sync.dma_start(lrow_all[:], len_ap.rearrange("(one b) -> one b", one=1))

    # prologue: per-tile +/-BIG softmax masks and empty-row gate masks
    ltf_all = wpool.tile([P, N_TILES], F32)
    nc.vector.tensor_copy(ltf_all[:], lt_all[:])
    maskbig_all = wpool.tile([P, N_TILES, M], F32)
    negrow_all = wpool.tile([1, BC], BF16)
    lrowf_all = wpool.tile([1, BC], F32)
    nc.vector.tensor_copy(lrowf_all[:], lrow_all[:])
    nc.vector.tensor_scalar(negrow_all[:], lrowf_all[:], 0.0, None, Alu.is_gt)
    nc.vector.tensor_scalar(negrow_all[:], negrow_all[:], BIG, BIG, Alu.mult, Alu.subtract)
    for t in range(N_TILES):
        mt_ = caps[t]
        nc.vector.tensor_scalar(
            maskbig_all[:, t, 0:mt_], iota_m[:, 0:mt_], ltf_all[:, t:t + 1],
            None, Alu.is_lt)
        nc.vector.tensor_scalar(
            maskbig_all[:, t, 0:mt_], maskbig_all[:, t, 0:mt_], 2.0 * BIG, BIG,
            Alu.mult, Alu.subtract)

    def scores_front(t):
        """DMA load (packed int8 rows + scales), cast, scores, masked
        softmax, h-transpose."""
        b0 = t * P
        MT = caps[t]
        mq = memq_pool.tile([P, M, D], I8, tag="memq")
        nc.sync.dma_start(
            mq[:, 0:MT, :],
            mem_ap[offs[t]:offs[t + 1], :].rearrange("(p m) d -> p m d", p=P),
        )
        sc16 = small.tile([P, M], F16, tag="sc16")
        nc.sync.dma_start(
            sc16[:, 0:MT],
            sc_ap[offs[t]:offs[t + 1]].rearrange("(p m) -> p m", p=P),
        )
        sc = small.tile([P, M], F32, tag="sc")
        nc.vector.tensor_copy(sc[:, 0:MT], sc16[:, 0:MT])
        # cast q -> bf16 (integers <= 127 are exact in bf16)
        mck = mem_pool.tile([P, M, D], BF16, tag="mem")
        nc.vector.tensor_copy(mck[:, 0:MT, :], mq[:, 0:MT, :])

        ht = h_all[:, t, :]

        # ---- scores[b, m] = <q[b, m, :], h[b, :]> (fused mult+accum) ----
        scratch = small.tile([P, D], BF16, tag="scratch")
        S = small.tile([P, M], F32, tag="S")
        for m in range(MT):
            nc.vector.scalar_tensor_tensor(
                out=scratch[:], in0=mck[:, m, :], scalar=1.0, in1=ht,
                op0=Alu.mult, op1=Alu.mult, accum_out=S[:, m:m + 1],
            )
        # fold the per-row dequant scale into the scores (pre-softmax)
        nc.vector.tensor_tensor(S[:, 0:MT], S[:, 0:MT], sc[:, 0:MT], Alu.mult)

        # ---- masked softmax over m: Sm = min(S, +/-BIG mask) ----
        Sm = small.tile([P, M], F32, tag="Sm")
        nc.vector.tensor_tensor(Sm[:, 0:MT], S[:, 0:MT], maskbig_all[:, t, 0:MT], Alu.min)
        negmax = small.tile([P, 1], F32, tag="negmax")
        nc.vector.tensor_reduce(negmax[:], Sm[:, 0:MT], AX.X, Alu.max, negate=True)
        E = xstage.tile([P, M], F32, tag="E")
        ssum = small.tile([P, 1], F32, tag="ssum")
        # ScalarE accumulator emits the softmax denominator with the exp
        nc.scalar.activation(E[:, 0:MT], Sm[:, 0:MT], Act.Exp, bias=negmax[:],
                             scale=1.0, accum_out=ssum[:])
        rinv = small.tile([P, 1], F32, tag="rinv")
        nc.vector.reciprocal(rinv[:], ssum[:])
        # fold the dequant scale into the attention weights for r = attn @ mem
        nc.vector.tensor_tensor(E[:, 0:MT], E[:, 0:MT], sc[:, 0:MT], Alu.mult)

        # h transpose (only needs ht)
        pt_h = ps1.tile([P, 2, P], BF16, tag="pth")
        hT = xstage.tile([P, 2, P], BF16, tag="hT")
        for k in range(2):
            nc.tensor.transpose(pt_h[:, k, :], ht[:, k * P:(k + 1) * P], ident16[:])
            nc.scalar.copy(hT[:, k, :], pt_h[:, k, :])

        return dict(ht=ht, hT=hT, negrow=negrow_all[:, b0:b0 + P],
                    attn=E, rinv=rinv, mck=mck, b0=b0, MT=MT,
                    last=(t >= N_TILES - 2))

    def r_front(st):
        """r[b, :] = sum_m attn'[b, m] * q[b, m, :], on TensorE via
        diag(attn'_m) bf16 matmuls accumulated in PSUM."""
        attn, mck, MT, last = st["attn"], st["mck"], st["MT"], st["last"]
        R_ps = ps.tile([P, D], F32, tag="Rps")
        for m in range(MT):
            dg = diag_pool.tile([P, P], BF16, tag="dg")
            if last and m % 3 != 0:
                nc.vector.tensor_scalar(dg[:], ident[:], attn[:, m:m + 1], None, Alu.mult)
            else:
                nc.scalar.activation(dg[:], ident[:], Act.Copy, bias=0.0,
                                     scale=attn[:, m:m + 1])
            nc.tensor.matmul(
                R_ps[:], dg[:], mck[:, m, :],
                start=(m == 0), stop=(m == MT - 1),
            )
        st["R_ps"] = R_ps
        return st

    def backend(st):
        """Combine r, gate matmuls, sigmoid, blend, store."""
        ht, R_ps, hT, negrow, b0 = (
            st["ht"], st["R_ps"], st["hT"], st["negrow"], st["b0"]
        )
        R = small.tile([P, D], F32, tag="R")
        nc.scalar.activation(R[:], R_ps[:], Act.Copy, bias=0.0, scale=st["rinv"][:])
        Rb = small.tile([P, D], BF16, tag="Rb")
        nc.vector.tensor_copy(Rb[:], R[:])

        pt_r = ps1.tile([P, 2, P], BF16, tag="ptr")
        rT = small.tile([P, 2, P], BF16, tag="rT")
        for k in range(2):
            nc.tensor.transpose(pt_r[:, k, :], Rb[:, k * P:(k + 1) * P], ident16[:])
            nc.scalar.copy(rT[:, k, :], pt_r[:, k, :])

        # ---- gate preactivation directly in [b, o] layout ----
        # G[b, o] = sum_d hT[d, b] Wg^T[d, o] + sum_d rT[d, b] Ug^T[d, o]
        #           + bias[o] + (-BIG if len_b == 0)
        # (contraction over d: lhsT = hT/rT blocks, rhs = wgT/ugT blocks;
        #  bias and empty-row mask enter as rank-1 matmuls)
        G = ps.tile([P, D], F32, tag="G")
        for ib in range(2):
            nc.tensor.matmul(G[:], hT[:, ib, :], wgT[:, ib, :],
                             start=(ib == 0), stop=False)
        for ib in range(2):
            nc.tensor.matmul(G[:], rT[:, ib, :], ugT[:, ib, :],
                             start=False, stop=False)
        nc.tensor.matmul(G[:], ones_col[:], bias_row[:], start=False, stop=False)
        nc.tensor.matmul(G[:], negrow[:], ones_D[:], start=False, stop=True)

        # y = tanh((pre + bias)/2); g = 0.5*(1+y) folded into the blend.
        g_sb = small.tile([P, D], F32, tag="gT")
        nc.scalar.activation(g_sb[:], G[:], Act.Tanh, bias=0.0, scale=0.5)

        # ---- out = h + 0.5*(1+y)*(r-h) ----
        T1 = small.tile([P, D], F32, tag="T1")
        nc.vector.tensor_tensor(T1[:], R[:], ht, Alu.subtract)
        T2 = small.tile([P, D], F32, tag="T2")
        nc.vector.scalar_tensor_tensor(
            out=T2[:], in0=g_sb[:], scalar=1.0,
            in1=T1[:], op0=Alu.add, op1=Alu.mult,
        )
        ot = out_pool.tile([P, D], BF16, tag="ot")
        nc.vector.scalar_tensor_tensor(
            out=ot[:], in0=T2[:], scalar=0.5, in1=ht, op0=Alu.mult, op1=Alu.add,
        )
        nc.sync.dma_start(out_ap[b0:b0 + P, :], ot[:])

    # 3-stage software pipeline.
    stages = []
    for t in range(N_TILES):
        stages.append(scores_front(t))
        if t >= 1:
            r_front(stages[t - 1])
        if t >= 2:
            backend(stages[t - 2])
    r_front(stages[N_TILES - 1])
    backend(stages[N_TILES - 2])
    backend(stages[N_TILES - 1])


_CACHE = {}


def _get_nc(caps):
    key = ("nc", caps)
    if key in _CACHE:
        return _CACHE[key]
    total_rows = int(P * sum(caps))
    nc = bacc.Bacc("TRN2", target_bir_lowering=False, debug=False, num_devices=N_CORES)
    h_ap = nc.dram_tensor("h_tilde", [BC, D], BF16, kind="ExternalInput").ap()
    mem_ap = nc.dram_tensor("mem", [total_rows, D], I8, kind="ExternalInput").ap()
    sc_ap = nc.dram_tensor("scales", [total_rows], F16, kind="ExternalInput").ap()
    len_ap = nc.dram_tensor("lengths", [BC], I32, kind="ExternalInput").ap()
    wg_ap = nc.dram_tensor("Wg_w", [D, D], BF16, kind="ExternalInput").ap()
    wgb_ap = nc.dram_tensor("Wg_b", [D], F32, kind="ExternalInput").ap()
    ug_ap = nc.dram_tensor("Ug_w", [D, D], BF16, kind="ExternalInput").ap()
    ugb_ap = nc.dram_tensor("Ug_b", [D], F32, kind="ExternalInput").ap()
    bg_ap = nc.dram_tensor("b_g", [D], F32, kind="ExternalInput").ap()
    out_ap = nc.dram_tensor("out", [BC, D], BF16, kind="ExternalOutput").ap()
    io = (h_ap, mem_ap, sc_ap, len_ap, wg_ap, wgb_ap, ug_ap, ugb_ap, bg_ap,
          out_ap)
    with tile.TileContext(nc) as tc:
        with ExitStack() as ctx:
            _build_body(ctx, tc, io, caps)
    nc.finalize()
    _CACHE[key] = nc
    return nc


def _plan(lengths):
    """Sort samples by length; deal global octile blocks across cores so
    every core has the same per-tile cap profile. Returns (perm[B] of
    sample ids in device order core-major, caps[N_TILES])."""
    order = np.argsort(lengths, kind="stable")
    caps = []
    perm = np.empty(B, dtype=np.int64)
    for k in range(N_TILES):
        blk = order[k * (P * N_CORES):(k + 1) * (P * N_CORES)]
        caps.append(int(max(1, lengths[blk].max())))
        # core c, tile k, partition p <- blk[p * N_CORES + c]
        for c in range(N_CORES):
            perm[c * BC + k * P: c * BC + (k + 1) * P] = blk[c::N_CORES]
    return perm, tuple(caps)


def _make_in_maps(inputs):
    lengths_full = np.asarray(inputs["lengths"], dtype=np.int32)
    perm, caps = _plan(lengths_full)
    h = np.asarray(inputs["h_tilde"], dtype=np.float32).astype(ml_dtypes.bfloat16)
    mem = np.asarray(inputs["mem"])
    shared = {
        "Wg_w": np.asarray(inputs["Wg_w"], dtype=np.float32).astype(ml_dtypes.bfloat16),
        "Wg_b": np.ascontiguousarray(np.asarray(inputs["Wg_b"], dtype=np.float32)),
        "Ug_w": np.asarray(inputs["Ug_w"], dtype=np.float32).astype(ml_dtypes.bfloat16),
        "Ug_b": np.ascontiguousarray(np.asarray(inputs["Ug_b"], dtype=np.float32)),
        "b_g": np.ascontiguousarray(np.asarray(inputs["b_g"], dtype=np.float32)),
    }
    total_rows = int(P * sum(caps))

    in_maps = []
    for c in range(N_CORES):
        ids = perm[c * BC:(c + 1) * BC]
        q_parts = np.empty((total_rows, D), dtype=np.int8)
        s_parts = np.empty((total_rows,), dtype=np.float16)
        off = 0
        for k in range(N_TILES):
            n = P * caps[k]
            rows = mem[ids[k * P:(k + 1) * P], :caps[k], :].reshape(n, D)
            amax = np.maximum(rows.max(axis=1), -rows.min(axis=1))
            np.maximum(amax, 1e-30, out=amax)
            s_parts[off:off + n] = amax * (1.0 / 127.0)
            np.multiply(rows, (127.0 / amax)[:, None], out=rows)
            np.rint(rows, out=q_parts[off:off + n], casting="unsafe")
            off += n
        in_maps.append({
            "h_tilde": np.ascontiguousarray(h[ids]),
            "mem": q_parts,
            "scales": s_parts,
            "lengths": np.ascontiguousarray(lengths_full[ids]),
            **shared,
        })
    return in_maps, perm, caps


def run(inputs, **kwargs):
    in_maps, perm, caps = _make_in_maps(inputs)
    nc = _get_nc(caps)
    res = run_bass_kernel_spmd(nc, in_maps, list(range(N_CORES)), **kwargs)
    return res, perm


def kernel(**inputs) -> np.ndarray:
    res, perm = run(inputs)
    permuted = np.concatenate(
        [res.results[c]["out"] for c in range(N_CORES)], axis=0)
    out = np.empty((B, D), dtype=np.float32)
    out[perm] = permuted
    return out
